# revision 8
# baseline (speedup 1.0000x reference)
"""Self-contained TRN2 Bass kernel for the GAT sublayer problem
(nn_GATSubLayer_26998164423437).

Strategy: dst-bucketed edge-parallel across 8 NeuronCores, no collectives,
no device-side gathers. Host sorts edges by destination, buckets per core /
per 128-node dst window, and materializes per-edge-slot tables:
hTe = h.T[:, src[slot]] (bf16 features) plus bf16 attention-score streams
s_src[slot], s_dst[slot] and rel-coef relv[slot]. The device STREAMS hTe
with large contiguous DMAs and computes z per 128-edge tile as
z = hTe^T @ W directly in PSUM (4 tiles packed per PSUM bank, one
batched PSUM->SBUF copy per pack). Per 4-window group the softmax
logits / exp / scale and the one-hot edge->dst-col matrices are computed
with batched strided ops spread across DVE/Act/Pool; the weighted
segment-sum + softmax denominator accumulate on the PE as
pacc = O^T @ [exp | w*z], finished by a 1/denom scale.
"""

import numpy as np
import jax
import jax.numpy as jnp
from jax.sharding import Mesh, PartitionSpec
from jax.experimental.shard_map import shard_map

import concourse.bass as bass
import concourse.mybir as mybir
import concourse.tile as _tile
from concourse.tile import TileContext
from concourse.bass2jax import (
    _bass_exec_p,
    install_neuronx_cc_hook,
    partition_id_tensor,
    fast_dispatch_compile,
)

N_CORES = 8

"""Patches for this walrus build.

The stock tail drain aggregates every live proc-semaphore wait onto a single
Drain instruction (bypassing bass's per-instruction wait-count validation);
walrus's CoreV3 setupSyncWait then rejects it ("Too many sync wait commands").
Emit one Drain per semaphore wait instead. DMA lane semaphores count 16 per
completed DMA, so their wait value is tick*16.
"""


def _split_drain_and_barrier(self, tick_clock, wait_clock):
    nc = self.nc
    clock = tick_clock.global_clock
    sems = wait_clock.sems
    pending = [(proc, tick) for proc, tick in enumerate(clock) if tick > 0]
    if not pending:
        nc.sync.drain()
    for proc, tick in pending:
        sem = sems[proc]
        val = tick * 16 if "DMA" in sem.name else tick
        nc.sync.drain().wait_op(sem, val, "sem-ge")
    nc.all_engine_barrier()
    assert self.sems is not None
    popped = nc._tile_sem_poison_stack.pop()
    assert popped is self._sem_poison
    nc.clear_and_free_semaphores(list(self.sems.allocated().values()))
    nc.all_engine_barrier()


_tile.TileContext._drain_and_barrier = _split_drain_and_barrier


def split_excess_waits(nc, max_cmds=2):
    """This walrus build allows at most 2 sync commands (waits + updates) per
    instruction. Tile's wait assignment can exceed that; peel extra waits onto
    EventSemaphore carriers (2 waits each) inserted just before the
    instruction on the same engine."""
    import concourse.mybir as mybir

    f = nc.m.functions[0]
    n_split = 0
    for bb in f.blocks:
        il = bb.instructions
        i = 0
        while i < len(il):
            ins = il[i]
            si = ins.sync_info
            if si is None:
                i += 1
                continue
            waits = list(si.on_wait or [])
            ups = list(si.on_update or [])
            budget = max(max_cmds - len(ups), 0)
            if len(waits) <= budget:
                i += 1
                continue
            keep = waits[:budget]
            extra = waits[budget:]
            ins.sync_info = mybir.SyncInfo(on_wait=keep, on_update=ups)
            carriers = []
            for j in range(0, len(extra), max_cmds):
                n_split += 1
                carriers.append(
                    mybir.InstEventSemaphore(
                        name=f"waitsplit_{n_split}",
                        engine=ins.engine,
                        sync_info=mybir.SyncInfo(
                            on_wait=extra[j : j + max_cmds], on_update=[]
                        ),
                    )
                )
            il[i:i] = carriers
            i += len(carriers) + 1
    return n_split


P = 128
D = 128
SROW = 129   # per-edge row: [exp slot, z(128)]
ZPK = 4      # z tiles packed per PSUM bank (4*128*4B = 2KB)
WG = 4       # windows per batched group


def host_prep(h, W, attn, rel_emb, src, dst, etype, n_cores, dt16=True):
    """Returns (in_maps, meta). All numpy."""
    N = h.shape[0]
    E = src.shape[0]
    npc = N // n_cores  # nodes per core
    assert npc * n_cores == N
    nwin = (npc + P - 1) // P

    wl = (W @ attn[:D]).astype(np.float32)
    wr = (W @ attn[D:]).astype(np.float32)
    s_src_n = (h @ wl).astype(np.float32)   # [N]
    s_dst_n = (h @ wr).astype(np.float32)   # [N]

    rel_table = rel_emb[:, 0].astype(np.float32).copy()
    rel_table[0] = 0.0
    relv_all = rel_table[etype]

    # ---- bucket edges by (core, window) ----
    core_of = dst // npc
    win_of = (dst % npc) // P
    key = core_of * nwin + win_of
    order = np.argsort(key, kind="stable")
    src_s, dst_s, relv_s, key_s = src[order], dst[order], relv_all[order], key[order]
    counts = np.bincount(key_s, minlength=n_cores * nwin).reshape(n_cores, nwin)
    # per-window tile count = max over cores (same compiled program everywhere)
    tiles_w = np.maximum((counts.max(axis=0) + P - 1) // P, 1).astype(np.int64)
    TT = int(tiles_w.sum())

    in_maps = []
    bounds = np.concatenate([[0], np.cumsum(counts.reshape(-1))])
    iotaR = np.tile(np.arange(P, dtype=np.float32)[None, :], (P, 1))  # [128,128]
    if dt16:
        import ml_dtypes
        xdt = ml_dtypes.bfloat16
    else:
        xdt = np.float32
    hT = np.ascontiguousarray(h.T).astype(xdt)  # [D, N]
    for c in range(n_cores):
        srcidx = np.zeros((TT * P,), np.int64)
        ssrc = np.zeros((P, TT), np.float32)
        sdst = np.zeros((P, TT), np.float32)
        dstcol = np.full((P, TT), -1.0, np.float32)
        relv = np.zeros((P, TT), np.float32)
        toff = 0
        for w in range(nwin):
            k = c * nwin + w
            s, e = bounds[k], bounds[k + 1]
            cnt = e - s
            Tw = int(tiles_w[w])
            # window's edges, padded to Tw*128
            se = np.zeros((Tw * P,), np.int64)
            se[:cnt] = src_s[s:e]
            ss = np.zeros((Tw * P,), np.float32)
            ss[:cnt] = s_src_n[src_s[s:e]]
            sd = np.zeros((Tw * P,), np.float32)
            sd[:cnt] = s_dst_n[dst_s[s:e]]
            dc = np.full((Tw * P,), -1.0, np.float32)
            dc[:cnt] = (dst_s[s:e] - c * npc - w * P).astype(np.float32)
            rv = np.zeros((Tw * P,), np.float32)
            rv[:cnt] = relv_s[s:e]
            # edge slot g*128+p -> partition p of tile col toff+g
            srcidx[(toff) * P : (toff + Tw) * P] = se
            ssrc[:, toff : toff + Tw] = ss.reshape(Tw, P).T
            sdst[:, toff : toff + Tw] = sd.reshape(Tw, P).T
            dstcol[:, toff : toff + Tw] = dc.reshape(Tw, P).T
            relv[:, toff : toff + Tw] = rv.reshape(Tw, P).T
            toff += Tw
        # per-edge-slot feature table, tile-major: col t*128+p = slot (t,p)
        hTe = np.ascontiguousarray(hT[:, srcidx])
        in_maps.append(
            {
                "hTe": hTe,
                "wW": np.ascontiguousarray(W).astype(xdt),
                "ssrc": ssrc.astype(xdt),
                "sdst": sdst.astype(xdt),
                "dstcol": dstcol.astype(xdt),
                "relv": relv.astype(xdt),
                "iota": iotaR.astype(xdt),
            }
        )
    meta = {
        "N": N,
        "npc": npc,
        "nwin": nwin,
        "tiles_w": [int(t) for t in tiles_w],
        "TT": TT,
        "n_cores": n_cores,
        "dt16": dt16,
    }
    return in_maps, meta


def build_kernel(meta, repeat=1, for_hw=True):
    nwin, TT = meta["nwin"], meta["TT"]
    tiles_w = meta["tiles_w"]
    f32 = mybir.dt.float32
    dt16 = meta.get("dt16", True)
    vdt = mybir.dt.bfloat16 if dt16 else f32

    nc = bass.Bass()
    hTe = nc.declare_dram_parameter("hTe", [D, TT * P], vdt, isOutput=False)
    wW = nc.declare_dram_parameter("wW", [D, D], vdt, isOutput=False)
    ssrc = nc.declare_dram_parameter("ssrc", [P, TT], vdt, isOutput=False)
    sdst = nc.declare_dram_parameter("sdst", [P, TT], vdt, isOutput=False)
    dstcol = nc.declare_dram_parameter("dstcol", [P, TT], vdt, isOutput=False)
    relv = nc.declare_dram_parameter("relv", [P, TT], vdt, isOutput=False)
    iota = nc.declare_dram_parameter("iota", [P, P], vdt, isOutput=False)
    out = nc.declare_dram_parameter("out", [nwin * P, D], f32, isOutput=True)

    # window -> batched groups of WG windows
    wgroups = []
    w = 0
    while w < nwin:
        wgroups.append(list(range(w, min(w + WG, nwin))))
        w += WG
    woff = np.concatenate([[0], np.cumsum(tiles_w)]).astype(int)

    with TileContext(nc) as tc:
        with (
            tc.tile_pool(name="const", bufs=1) as cpool,
            tc.tile_pool(name="feat", bufs=2) as fpool,
            tc.tile_pool(name="zps", bufs=4, space="PSUM") as zpsum,
            tc.tile_pool(name="ewin", bufs=2) as wpool,
            tc.tile_pool(name="eps", bufs=2, space="PSUM") as epsum,
        ):
            wWsb = cpool.tile([D, D], vdt, tag="wW")
            nc.sync.dma_start(out=wWsb[:], in_=wW[:])
            iotasb = cpool.tile([P, P], vdt, tag="iota")
            nc.sync.dma_start(out=iotasb[:], in_=iota[:])
            ssrcsb = cpool.tile([P, TT], vdt, tag="ssrc")
            nc.sync.dma_start(out=ssrcsb[:], in_=ssrc[:])
            sdstsb = cpool.tile([P, TT], vdt, tag="sdst")
            nc.sync.dma_start(out=sdstsb[:], in_=sdst[:])
            dcolsb = cpool.tile([P, TT], vdt, tag="dstcol")
            nc.sync.dma_start(out=dcolsb[:], in_=dstcol[:])
            relvsb = cpool.tile([P, TT], vdt, tag="relv")
            nc.sync.dma_start(out=relvsb[:], in_=relv[:])

            copy_engines = [nc.scalar, nc.vector]
            ci = 0
            for _rep in range(repeat):
                for grp in wgroups:
                    toff = int(woff[grp[0]])
                    gTw = int(woff[grp[-1] + 1] - woff[grp[0]])
                    # stream this group's per-edge-slot features
                    he = fpool.tile([D, gTw * P], vdt, tag="he")
                    nc.sync.dma_start(
                        out=he[:], in_=hTe[:, toff * P : (toff + gTw) * P]
                    )
                    zs = wpool.tile([P, gTw * SROW], vdt, tag="zs")
                    zv = zs[:].rearrange("p (t c) -> p t c", c=SROW)
                    for t0 in range(0, gTw, ZPK):
                        kl = min(ZPK, gTw - t0)
                        zp = zpsum.tile([P, kl * D], f32, tag="zp")
                        for j in range(kl):
                            nc.tensor.matmul(
                                out=zp[:, j * D : (j + 1) * D],
                                lhsT=he[:, (t0 + j) * P : (t0 + j + 1) * P],
                                rhs=wWsb[:],
                                start=True,
                                stop=True,
                            )
                        eng = copy_engines[ci % len(copy_engines)]
                        ci += 1
                        if eng is nc.scalar:
                            eng.copy(
                                out=zv[:, t0 : t0 + kl, 1 : 1 + D],
                                in_=zp[:].rearrange("p (k c) -> p k c", c=D),
                            )
                        else:
                            eng.tensor_copy(
                                out=zv[:, t0 : t0 + kl, 1 : 1 + D],
                                in_=zp[:].rearrange("p (k c) -> p k c", c=D),
                            )
                    # x = s_src + s_dst ; leaky relu = max(x, 0.01x)
                    xw = wpool.tile([P, gTw], f32, tag="xw")
                    nc.vector.tensor_tensor(
                        out=xw[:], in0=ssrcsb[:, toff : toff + gTw],
                        in1=sdstsb[:, toff : toff + gTw],
                        op=mybir.AluOpType.add,
                    )
                    xs = wpool.tile([P, gTw], f32, tag="xs")
                    nc.vector.tensor_scalar(
                        out=xs[:], in0=xw[:], scalar1=0.01, scalar2=None,
                        op0=mybir.AluOpType.mult,
                    )
                    nc.vector.tensor_tensor(
                        out=xw[:], in0=xw[:], in1=xs[:], op=mybir.AluOpType.max
                    )
                    # exp into the leading slot of each row
                    nc.scalar.activation(
                        out=zv[:, :, 0], in_=xw[:],
                        func=mybir.ActivationFunctionType.Exp,
                    )
                    wexp = wpool.tile([P, gTw], vdt, tag="wexp")
                    nc.vector.tensor_tensor(
                        out=wexp[:], in0=zv[:, :, 0],
                        in1=relvsb[:, toff : toff + gTw],
                        op=mybir.AluOpType.mult,
                    )
                    # scale z columns by wexp (batched, strided, on gpsimd)
                    nc.gpsimd.tensor_tensor(
                        out=zv[:, :, 1 : 1 + D],
                        in0=zv[:, :, 1 : 1 + D],
                        in1=wexp[:].rearrange("p (t o) -> p t o", o=1).to_broadcast(
                            [P, gTw, D]
                        ),
                        op=mybir.AluOpType.mult,
                    )
                    # one-hot edge -> dst col matrices (batched)
                    ob = wpool.tile([P, gTw * P], vdt, tag="ob")
                    nc.vector.tensor_tensor(
                        out=ob[:].rearrange("p (t c) -> p t c", c=P),
                        in0=dcolsb[:, toff : toff + gTw].to_broadcast([P, gTw, P]),
                        in1=iotasb[:].rearrange("p (o c) -> p o c", o=1).to_broadcast(
                            [P, gTw, P]
                        ),
                        op=mybir.AluOpType.is_equal,
                    )
                    for w in grp:
                        Tw = tiles_w[w]
                        t0 = int(woff[w]) - toff
                        pacc = epsum.tile([P, SROW], f32, tag="pacc")
                        for t in range(t0, t0 + Tw):
                            nc.tensor.matmul(
                                out=pacc[:],
                                lhsT=ob[:, t * P : (t + 1) * P],
                                rhs=zs[:, t * SROW : (t + 1) * SROW],
                                start=(t == t0),
                                stop=(t == t0 + Tw - 1),
                            )
                        dn = wpool.tile([P, 1], f32, tag="dn")
                        nc.vector.tensor_scalar(
                            out=dn[:], in0=pacc[:, 0:1], scalar1=1e-30,
                            scalar2=None, op0=mybir.AluOpType.max,
                        )
                        rec = wpool.tile([P, 1], f32, tag="rec")
                        nc.vector.reciprocal(out=rec[:], in_=dn[:])
                        ow = wpool.tile([P, D], f32, tag="ow")
                        nc.scalar.mul(out=ow[:], in_=pacc[:, 1 : 1 + D], mul=rec[:, 0:1])
                        nc.sync.dma_start(
                            out=out[w * P : (w + 1) * P, :], in_=ow[:]
                        )
    if for_hw:
        split_excess_waits(nc)
    return nc


def ref_numpy(h, W, attn, rel_emb, src, dst, etype):
    rel_table = rel_emb.copy()
    rel_table[0] = 0.0
    z = h @ W
    s_src = z @ attn[: W.shape[1]]
    s_dst = z @ attn[W.shape[1] :]
    N = h.shape[0]
    x = s_src[src] + s_dst[dst]
    e = np.where(x > 0, x, 0.01 * x)
    ex = np.exp(e)
    denom = np.zeros(N)
    np.add.at(denom, dst, ex)
    alpha = ex / denom[dst]
    coef = rel_table[etype, 0] * alpha
    out = np.zeros((N, W.shape[1]), np.float64)
    np.add.at(out, dst, coef[:, None] * z[src])
    return out.astype(np.float32)


def make_runner(nc: bass.Bass, in_maps, n_cores: int, chain: int = 1):
    install_neuronx_cc_hook()
    assert nc.dbg_addr is None or not nc.dbg_callbacks

    partition_name = nc.partition_id_tensor.name if nc.partition_id_tensor else None
    in_names, out_names, out_avals = [], [], []
    for alloc in nc.m.functions[0].allocations:
        if not isinstance(alloc, mybir.MemoryLocationSet):
            continue
        name = alloc.memorylocations[0].name
        if alloc.kind == "ExternalInput":
            if name != partition_name and name != (nc.dbg_addr.name if nc.dbg_addr else None):
                in_names.append(name)
        elif alloc.kind == "ExternalOutput":
            out_names.append(name)
            out_avals.append(
                jax.core.ShapedArray(tuple(alloc.tensor_shape), mybir.dt.np(alloc.dtype))
            )
    n_params = len(in_names)
    all_in_names = list(in_names) + list(out_names)
    if nc.dbg_addr is not None:
        in_maps = [{**m, nc.dbg_addr.name: np.zeros((1, 2), np.uint32)} for m in in_maps]
        all_in_names.insert(n_params, nc.dbg_addr.name)  # keep order consistent w/ alloc?
    if partition_name is not None:
        all_in_names.append(partition_name)

    def _body(*args):
        operands = list(args)
        if partition_name is not None:
            operands.append(partition_id_tensor())
        outs = _bass_exec_p.bind(
            *operands,
            out_avals=tuple(out_avals),
            in_names=tuple(all_in_names),
            out_names=tuple(out_names),
            lowering_input_output_aliases=(),
            sim_require_finite=True,
            sim_require_nnan=True,
            nc=nc,
        )
        return tuple(outs)

    devices = jax.devices()[:n_cores]
    mesh = Mesh(np.asarray(devices), ("core",))
    n_outs = len(out_names)

    def _chained(*args):
        params = args[: n_params]
        outs = args[n_params :]
        for _ in range(chain):
            outs = _body(*params, *outs)
        return outs

    def wrapper(*ins):
        return shard_map(
            _chained,
            mesh=mesh,
            in_specs=(PartitionSpec("core"),) * (n_params + n_outs),
            out_specs=(PartitionSpec("core"),) * n_outs,
            check_rep=False,
        )(*ins)

    sh = jax.sharding.NamedSharding(mesh, PartitionSpec("core"))
    concat_in = [
        jax.device_put(
            np.concatenate([np.asarray(in_maps[c][nm]) for c in range(n_cores)], axis=0),
            sh,
        )
        for nm in in_names
    ] + [
        jax.device_put(
            np.zeros((av.shape[0] * n_cores,) + tuple(av.shape[1:]), av.dtype), sh
        )
        for av in out_avals
    ]

    jitted = fast_dispatch_compile(
        lambda: jax.jit(wrapper).lower(*concat_in).compile()
    )

    def run():
        outs = jitted(*concat_in)
        jax.block_until_ready(outs)
        return outs

    def collect(outs):
        res = []
        for c in range(n_cores):
            d = {}
            for i, nm in enumerate(out_names):
                rows = out_avals[i].shape[0]
                d[nm] = np.asarray(outs[i][c * rows : (c + 1) * rows])
            res.append(d)
        return res

    return run, collect


def kernel(**inputs):
    inputs = {k: np.asarray(v) for k, v in inputs.items()}
    in_maps, meta = host_prep(**inputs, n_cores=N_CORES)
    nc = build_kernel(meta)
    run, collect = make_runner(nc, in_maps, N_CORES)
    res = collect(run())
    out = np.concatenate([res[c]["out"][: meta["npc"]] for c in range(N_CORES)], axis=0)
    return out.astype(np.float32)


# revision 9
# speedup vs baseline: 1.4764x; 1.4764x over previous
"""Self-contained TRN2 Bass kernel for the GAT sublayer problem
(nn_GATSubLayer_26998164423437).

Strategy: dst-bucketed edge-parallel across 8 NeuronCores, no collectives,
no device-side gathers. Host sorts edges by destination, buckets per core /
per 128-node dst window, and materializes per-edge-slot tables:
hTe = h.T[:, src[slot]] (bf16 features) plus bf16 attention-score streams
s_src[slot], s_dst[slot] and rel-coef relv[slot]. The device STREAMS hTe
with large contiguous DMAs and computes z per 128-edge tile as
z = hTe^T @ W directly in PSUM (4 tiles packed per PSUM bank, one
batched PSUM->SBUF copy per pack). Per 4-window group the softmax
logits / exp / scale and the one-hot edge->dst-col matrices are computed
with batched strided ops spread across DVE/Act/Pool; the weighted
segment-sum + softmax denominator accumulate on the PE as
pacc = O^T @ [exp | w*z], finished by a 1/denom scale.
"""

import numpy as np
import jax
import jax.numpy as jnp
from jax.sharding import Mesh, PartitionSpec
from jax.experimental.shard_map import shard_map

import concourse.bass as bass
import concourse.mybir as mybir
import concourse.tile as _tile
from concourse.tile import TileContext
from concourse.bass2jax import (
    _bass_exec_p,
    install_neuronx_cc_hook,
    partition_id_tensor,
    fast_dispatch_compile,
)

N_CORES = 8

"""Patches for this walrus build.

The stock tail drain aggregates every live proc-semaphore wait onto a single
Drain instruction (bypassing bass's per-instruction wait-count validation);
walrus's CoreV3 setupSyncWait then rejects it ("Too many sync wait commands").
Emit one Drain per semaphore wait instead. DMA lane semaphores count 16 per
completed DMA, so their wait value is tick*16.
"""


def _split_drain_and_barrier(self, tick_clock, wait_clock):
    nc = self.nc
    clock = tick_clock.global_clock
    sems = wait_clock.sems
    pending = [(proc, tick) for proc, tick in enumerate(clock) if tick > 0]
    if not pending:
        nc.sync.drain()
    for proc, tick in pending:
        sem = sems[proc]
        val = tick * 16 if "DMA" in sem.name else tick
        nc.sync.drain().wait_op(sem, val, "sem-ge")
    nc.all_engine_barrier()
    assert self.sems is not None
    popped = nc._tile_sem_poison_stack.pop()
    assert popped is self._sem_poison
    nc.clear_and_free_semaphores(list(self.sems.allocated().values()))
    nc.all_engine_barrier()


_tile.TileContext._drain_and_barrier = _split_drain_and_barrier


def split_excess_waits(nc, max_cmds=2):
    """This walrus build allows at most 2 sync commands (waits + updates) per
    instruction. Tile's wait assignment can exceed that; peel extra waits onto
    EventSemaphore carriers (2 waits each) inserted just before the
    instruction on the same engine."""
    import concourse.mybir as mybir

    f = nc.m.functions[0]
    n_split = 0
    for bb in f.blocks:
        il = bb.instructions
        i = 0
        while i < len(il):
            ins = il[i]
            si = ins.sync_info
            if si is None:
                i += 1
                continue
            waits = list(si.on_wait or [])
            ups = list(si.on_update or [])
            budget = max(max_cmds - len(ups), 0)
            if len(waits) <= budget:
                i += 1
                continue
            keep = waits[:budget]
            extra = waits[budget:]
            ins.sync_info = mybir.SyncInfo(on_wait=keep, on_update=ups)
            carriers = []
            for j in range(0, len(extra), max_cmds):
                n_split += 1
                carriers.append(
                    mybir.InstEventSemaphore(
                        name=f"waitsplit_{n_split}",
                        engine=ins.engine,
                        sync_info=mybir.SyncInfo(
                            on_wait=extra[j : j + max_cmds], on_update=[]
                        ),
                    )
                )
            il[i:i] = carriers
            i += len(carriers) + 1
    return n_split


P = 128
D = 128
SROW = 129   # per-edge row: [exp slot, z(128)]
ZPK = 4      # z tiles packed per PSUM bank (4*128*4B = 2KB)
WG = 4       # windows per batched group


def host_prep(h, W, attn, rel_emb, src, dst, etype, n_cores, dt16=True):
    """Returns (in_maps, meta). All numpy."""
    N = h.shape[0]
    E = src.shape[0]
    npc = N // n_cores  # nodes per core
    assert npc * n_cores == N
    nwin = (npc + P - 1) // P

    wl = (W @ attn[:D]).astype(np.float32)
    wr = (W @ attn[D:]).astype(np.float32)
    s_src_n = (h @ wl).astype(np.float32)   # [N]
    s_dst_n = (h @ wr).astype(np.float32)   # [N]

    rel_table = rel_emb[:, 0].astype(np.float32).copy()
    rel_table[0] = 0.0
    relv_all = rel_table[etype]

    # ---- bucket edges by (core, window) ----
    core_of = dst // npc
    win_of = (dst % npc) // P
    key = core_of * nwin + win_of
    order = np.argsort(key, kind="stable")
    src_s, dst_s, relv_s, key_s = src[order], dst[order], relv_all[order], key[order]
    counts = np.bincount(key_s, minlength=n_cores * nwin).reshape(n_cores, nwin)
    # per-window tile count = max over cores (same compiled program everywhere)
    tiles_w = np.maximum((counts.max(axis=0) + P - 1) // P, 1).astype(np.int64)
    TT = int(tiles_w.sum())

    in_maps = []
    bounds = np.concatenate([[0], np.cumsum(counts.reshape(-1))])
    iotaR = np.tile(np.arange(P, dtype=np.float32)[None, :], (P, 1))  # [128,128]
    if dt16:
        import ml_dtypes
        xdt = ml_dtypes.bfloat16
    else:
        xdt = np.float32
    hT = np.ascontiguousarray(h.T).astype(xdt)  # [D, N]
    for c in range(n_cores):
        srcidx = np.zeros((TT * P,), np.int64)
        ssrc = np.zeros((P, TT), np.float32)
        sdst = np.zeros((P, TT), np.float32)
        dstcol = np.full((P, TT), -1.0, np.float32)
        relv = np.zeros((P, TT), np.float32)
        toff = 0
        for w in range(nwin):
            k = c * nwin + w
            s, e = bounds[k], bounds[k + 1]
            cnt = e - s
            Tw = int(tiles_w[w])
            # window's edges, padded to Tw*128
            se = np.zeros((Tw * P,), np.int64)
            se[:cnt] = src_s[s:e]
            ss = np.zeros((Tw * P,), np.float32)
            ss[:cnt] = s_src_n[src_s[s:e]]
            sd = np.zeros((Tw * P,), np.float32)
            sd[:cnt] = s_dst_n[dst_s[s:e]]
            dc = np.full((Tw * P,), -1.0, np.float32)
            dc[:cnt] = (dst_s[s:e] - c * npc - w * P).astype(np.float32)
            rv = np.zeros((Tw * P,), np.float32)
            rv[:cnt] = relv_s[s:e]
            # edge slot g*128+p -> partition p of tile col toff+g
            srcidx[(toff) * P : (toff + Tw) * P] = se
            ssrc[:, toff : toff + Tw] = ss.reshape(Tw, P).T
            sdst[:, toff : toff + Tw] = sd.reshape(Tw, P).T
            dstcol[:, toff : toff + Tw] = dc.reshape(Tw, P).T
            relv[:, toff : toff + Tw] = rv.reshape(Tw, P).T
            toff += Tw
        # per-edge-slot feature table, tile-major: col t*128+p = slot (t,p)
        hTe = np.ascontiguousarray(hT[:, srcidx])
        in_maps.append(
            {
                "hTe": hTe,
                "wW": np.ascontiguousarray(W).astype(xdt),
                "ssrc": ssrc.astype(xdt),
                "sdst": sdst.astype(xdt),
                "dstcol": dstcol.astype(xdt),
                "relv": relv.astype(xdt),
                "iota": iotaR.astype(xdt),
            }
        )
    meta = {
        "N": N,
        "npc": npc,
        "nwin": nwin,
        "tiles_w": [int(t) for t in tiles_w],
        "TT": TT,
        "n_cores": n_cores,
        "dt16": dt16,
    }
    return in_maps, meta


def build_kernel(meta, repeat=1, for_hw=True):
    nwin, TT = meta["nwin"], meta["TT"]
    tiles_w = meta["tiles_w"]
    f32 = mybir.dt.float32
    dt16 = meta.get("dt16", True)
    vdt = mybir.dt.bfloat16 if dt16 else f32

    nc = bass.Bass()
    hTe = nc.declare_dram_parameter("hTe", [D, TT * P], vdt, isOutput=False)
    wW = nc.declare_dram_parameter("wW", [D, D], vdt, isOutput=False)
    ssrc = nc.declare_dram_parameter("ssrc", [P, TT], vdt, isOutput=False)
    sdst = nc.declare_dram_parameter("sdst", [P, TT], vdt, isOutput=False)
    dstcol = nc.declare_dram_parameter("dstcol", [P, TT], vdt, isOutput=False)
    relv = nc.declare_dram_parameter("relv", [P, TT], vdt, isOutput=False)
    iota = nc.declare_dram_parameter("iota", [P, P], vdt, isOutput=False)
    out = nc.declare_dram_parameter("out", [nwin * P, D], f32, isOutput=True)

    # window -> batched groups of WG windows
    wgroups = []
    w = 0
    while w < nwin:
        wgroups.append(list(range(w, min(w + WG, nwin))))
        w += WG
    woff = np.concatenate([[0], np.cumsum(tiles_w)]).astype(int)

    with TileContext(nc) as tc:
        with (
            tc.tile_pool(name="const", bufs=1) as cpool,
            tc.tile_pool(name="feat", bufs=2) as fpool,
            tc.tile_pool(name="zps", bufs=4, space="PSUM") as zpsum,
            tc.tile_pool(name="ewin", bufs=2) as wpool,
            tc.tile_pool(name="eps", bufs=2, space="PSUM") as epsum,
        ):
            wWsb = cpool.tile([D, D], vdt, tag="wW")
            nc.sync.dma_start(out=wWsb[:], in_=wW[:])
            iotasb = cpool.tile([P, P], vdt, tag="iota")
            nc.sync.dma_start(out=iotasb[:], in_=iota[:])
            ssrcsb = cpool.tile([P, TT], vdt, tag="ssrc")
            nc.sync.dma_start(out=ssrcsb[:], in_=ssrc[:])
            sdstsb = cpool.tile([P, TT], vdt, tag="sdst")
            nc.sync.dma_start(out=sdstsb[:], in_=sdst[:])
            dcolsb = cpool.tile([P, TT], vdt, tag="dstcol")
            nc.sync.dma_start(out=dcolsb[:], in_=dstcol[:])
            relvsb = cpool.tile([P, TT], vdt, tag="relv")
            nc.sync.dma_start(out=relvsb[:], in_=relv[:])

            copy_engines = [nc.scalar, nc.vector]
            ci = 0
            for _rep in range(repeat):
                for grp in wgroups:
                    toff = int(woff[grp[0]])
                    gTw = int(woff[grp[-1] + 1] - woff[grp[0]])
                    # stream this group's per-edge-slot features
                    he = fpool.tile([D, gTw * P], vdt, tag="he")
                    nc.sync.dma_start(
                        out=he[:], in_=hTe[:, toff * P : (toff + gTw) * P]
                    )
                    zs = wpool.tile([P, gTw * SROW], vdt, tag="zs")
                    zv = zs[:].rearrange("p (t c) -> p t c", c=SROW)
                    for t0 in range(0, gTw, ZPK):
                        kl = min(ZPK, gTw - t0)
                        zp = zpsum.tile([P, kl * D], f32, tag="zp")
                        for j in range(kl):
                            nc.tensor.matmul(
                                out=zp[:, j * D : (j + 1) * D],
                                lhsT=he[:, (t0 + j) * P : (t0 + j + 1) * P],
                                rhs=wWsb[:],
                                start=True,
                                stop=True,
                            )
                        eng = copy_engines[ci % len(copy_engines)]
                        ci += 1
                        if eng is nc.scalar:
                            eng.copy(
                                out=zv[:, t0 : t0 + kl, 1 : 1 + D],
                                in_=zp[:].rearrange("p (k c) -> p k c", c=D),
                            )
                        else:
                            eng.tensor_copy(
                                out=zv[:, t0 : t0 + kl, 1 : 1 + D],
                                in_=zp[:].rearrange("p (k c) -> p k c", c=D),
                            )
                    # x = s_src + s_dst ; leaky relu = max(x, 0.01x)
                    xw = wpool.tile([P, gTw], f32, tag="xw")
                    nc.vector.tensor_tensor(
                        out=xw[:], in0=ssrcsb[:, toff : toff + gTw],
                        in1=sdstsb[:, toff : toff + gTw],
                        op=mybir.AluOpType.add,
                    )
                    xs = wpool.tile([P, gTw], f32, tag="xs")
                    nc.vector.tensor_scalar(
                        out=xs[:], in0=xw[:], scalar1=0.01, scalar2=None,
                        op0=mybir.AluOpType.mult,
                    )
                    nc.vector.tensor_tensor(
                        out=xw[:], in0=xw[:], in1=xs[:], op=mybir.AluOpType.max
                    )
                    # exp into the leading slot of each row
                    nc.scalar.activation(
                        out=zv[:, :, 0], in_=xw[:],
                        func=mybir.ActivationFunctionType.Exp,
                    )
                    wexp = wpool.tile([P, gTw], vdt, tag="wexp")
                    nc.vector.tensor_tensor(
                        out=wexp[:], in0=zv[:, :, 0],
                        in1=relvsb[:, toff : toff + gTw],
                        op=mybir.AluOpType.mult,
                    )
                    # scale z columns by wexp (batched, strided)
                    nc.vector.tensor_tensor(
                        out=zv[:, :, 1 : 1 + D],
                        in0=zv[:, :, 1 : 1 + D],
                        in1=wexp[:].rearrange("p (t o) -> p t o", o=1).to_broadcast(
                            [P, gTw, D]
                        ),
                        op=mybir.AluOpType.mult,
                    )
                    # one-hot edge -> dst col matrices (batched)
                    ob = wpool.tile([P, gTw * P], vdt, tag="ob")
                    nc.vector.tensor_tensor(
                        out=ob[:].rearrange("p (t c) -> p t c", c=P),
                        in0=dcolsb[:, toff : toff + gTw].to_broadcast([P, gTw, P]),
                        in1=iotasb[:].rearrange("p (o c) -> p o c", o=1).to_broadcast(
                            [P, gTw, P]
                        ),
                        op=mybir.AluOpType.is_equal,
                    )
                    for w in grp:
                        Tw = tiles_w[w]
                        t0 = int(woff[w]) - toff
                        pacc = epsum.tile([P, SROW], f32, tag="pacc")
                        for t in range(t0, t0 + Tw):
                            nc.tensor.matmul(
                                out=pacc[:],
                                lhsT=ob[:, t * P : (t + 1) * P],
                                rhs=zs[:, t * SROW : (t + 1) * SROW],
                                start=(t == t0),
                                stop=(t == t0 + Tw - 1),
                            )
                        dn = wpool.tile([P, 1], f32, tag="dn")
                        nc.vector.tensor_scalar(
                            out=dn[:], in0=pacc[:, 0:1], scalar1=1e-30,
                            scalar2=None, op0=mybir.AluOpType.max,
                        )
                        rec = wpool.tile([P, 1], f32, tag="rec")
                        nc.vector.reciprocal(out=rec[:], in_=dn[:])
                        ow = wpool.tile([P, D], f32, tag="ow")
                        nc.scalar.mul(out=ow[:], in_=pacc[:, 1 : 1 + D], mul=rec[:, 0:1])
                        nc.sync.dma_start(
                            out=out[w * P : (w + 1) * P, :], in_=ow[:]
                        )
    if for_hw:
        split_excess_waits(nc)
    return nc


def ref_numpy(h, W, attn, rel_emb, src, dst, etype):
    rel_table = rel_emb.copy()
    rel_table[0] = 0.0
    z = h @ W
    s_src = z @ attn[: W.shape[1]]
    s_dst = z @ attn[W.shape[1] :]
    N = h.shape[0]
    x = s_src[src] + s_dst[dst]
    e = np.where(x > 0, x, 0.01 * x)
    ex = np.exp(e)
    denom = np.zeros(N)
    np.add.at(denom, dst, ex)
    alpha = ex / denom[dst]
    coef = rel_table[etype, 0] * alpha
    out = np.zeros((N, W.shape[1]), np.float64)
    np.add.at(out, dst, coef[:, None] * z[src])
    return out.astype(np.float32)


def make_runner(nc: bass.Bass, in_maps, n_cores: int, chain: int = 1):
    install_neuronx_cc_hook()
    assert nc.dbg_addr is None or not nc.dbg_callbacks

    partition_name = nc.partition_id_tensor.name if nc.partition_id_tensor else None
    in_names, out_names, out_avals = [], [], []
    for alloc in nc.m.functions[0].allocations:
        if not isinstance(alloc, mybir.MemoryLocationSet):
            continue
        name = alloc.memorylocations[0].name
        if alloc.kind == "ExternalInput":
            if name != partition_name and name != (nc.dbg_addr.name if nc.dbg_addr else None):
                in_names.append(name)
        elif alloc.kind == "ExternalOutput":
            out_names.append(name)
            out_avals.append(
                jax.core.ShapedArray(tuple(alloc.tensor_shape), mybir.dt.np(alloc.dtype))
            )
    n_params = len(in_names)
    all_in_names = list(in_names) + list(out_names)
    if nc.dbg_addr is not None:
        in_maps = [{**m, nc.dbg_addr.name: np.zeros((1, 2), np.uint32)} for m in in_maps]
        all_in_names.insert(n_params, nc.dbg_addr.name)  # keep order consistent w/ alloc?
    if partition_name is not None:
        all_in_names.append(partition_name)

    def _body(*args):
        operands = list(args)
        if partition_name is not None:
            operands.append(partition_id_tensor())
        outs = _bass_exec_p.bind(
            *operands,
            out_avals=tuple(out_avals),
            in_names=tuple(all_in_names),
            out_names=tuple(out_names),
            lowering_input_output_aliases=(),
            sim_require_finite=True,
            sim_require_nnan=True,
            nc=nc,
        )
        return tuple(outs)

    devices = jax.devices()[:n_cores]
    mesh = Mesh(np.asarray(devices), ("core",))
    n_outs = len(out_names)

    def _chained(*args):
        params = args[: n_params]
        outs = args[n_params :]
        for _ in range(chain):
            outs = _body(*params, *outs)
        return outs

    def wrapper(*ins):
        return shard_map(
            _chained,
            mesh=mesh,
            in_specs=(PartitionSpec("core"),) * (n_params + n_outs),
            out_specs=(PartitionSpec("core"),) * n_outs,
            check_rep=False,
        )(*ins)

    sh = jax.sharding.NamedSharding(mesh, PartitionSpec("core"))
    concat_in = [
        jax.device_put(
            np.concatenate([np.asarray(in_maps[c][nm]) for c in range(n_cores)], axis=0),
            sh,
        )
        for nm in in_names
    ] + [
        jax.device_put(
            np.zeros((av.shape[0] * n_cores,) + tuple(av.shape[1:]), av.dtype), sh
        )
        for av in out_avals
    ]

    jitted = fast_dispatch_compile(
        lambda: jax.jit(wrapper).lower(*concat_in).compile()
    )

    def run():
        outs = jitted(*concat_in)
        jax.block_until_ready(outs)
        return outs

    def collect(outs):
        res = []
        for c in range(n_cores):
            d = {}
            for i, nm in enumerate(out_names):
                rows = out_avals[i].shape[0]
                d[nm] = np.asarray(outs[i][c * rows : (c + 1) * rows])
            res.append(d)
        return res

    return run, collect


def kernel(**inputs):
    inputs = {k: np.asarray(v) for k, v in inputs.items()}
    in_maps, meta = host_prep(**inputs, n_cores=N_CORES)
    nc = build_kernel(meta)
    run, collect = make_runner(nc, in_maps, N_CORES)
    res = collect(run())
    out = np.concatenate([res[c]["out"][: meta["npc"]] for c in range(N_CORES)], axis=0)
    return out.astype(np.float32)
